# revision 1
# baseline (speedup 1.0000x reference)
"""Trainium2 Bass kernel for nn_Attention (general-score attention with
masked softmax), data-parallel over batch across 8 NeuronCores.

Math (per batch), matching the reference exactly for {0,1} float masks:
    raw[t,s]  = sum_e (hidden @ W)[t,e] * enc[s,e]       (associativity trick:
                (hidden @ W) @ enc^T  ==  hidden @ (enc @ W^T)^T, saves 25%
                FLOPs and avoids materializing proj)
    attn_energies = raw * mask            (mask in {0,1} so mask^2 == mask)
    e = exp(x - max_s x) * mask
    attn = e / (sum_s e + 1e-6)
    context = attn @ enc_value

Layouts: host marshals hidden^T (D,T) and enc^T (E,S) per batch so every
matmul contracts over the partition dim with zero on-device transposes,
except attn^T which is produced on-device via PE transpose (bf16).
mm1/mm2 run in float32r (e8m11; 1 cycle/row at N>=512 vs 4 for plain f32)
to keep the softmax exponents accurate; the attention tail (attn, val, mm3)
runs in bf16. Outputs ae/ctx/aw are rounded to bf16 on device and widened
to f32 on the host. Measured end-to-end rel err ~2.4e-3.

Schedule (two batches per core, software-pipelined):
  b0: loads -> mm1 (dt-outer over 8 psum banks, consumes DMA slices as they
      land) -> mm2 for all 4 t-tiles (groups kept sequential with explicit
      deps; softmax chains pipeline behind on DVE/ACT) -> per t-tile PE
      transposes + mm3, with b1's mm1 (et-outer, psB halves) interleaved
      between t-tiles so the PE never waits on a softmax chain.
  b1: same minus the interleaved successor.
"""
import os

import ml_dtypes
import numpy as np

B, TRG, SRC, ENCD, TRGD = 16, 512, 1024, 1024, 1024
NCORES = 8
BPC = B // NCORES  # batches per core
P = 128
nD = TRGD // P   # 8 contraction tiles over d
nE = ENCD // P   # 8 over e
nS = SRC // P    # 8 over s
nT = TRG // P    # 4 t-tiles

_cache = {}

LAST_EXEC_NS = None
LAST_RESULTS = None


def _build():
    import bass_rust
    import concourse.mybir as mybir
    import concourse.tile as tile
    from concourse import bacc
    from concourse.masks import make_identity

    _add_dep = bass_rust.add_dep_helper

    F32 = mybir.dt.float32
    F32R = mybir.dt.float32r
    BF16 = mybir.dt.bfloat16
    ALU = mybir.AluOpType
    AXL = mybir.AxisListType
    ACT_EXP = mybir.ActivationFunctionType.Exp

    nc = bacc.Bacc("TRN2", target_bir_lowering=False, debug=False)

    hidT_d = nc.dram_tensor("hidT", (BPC, TRGD, TRG), F32R, kind="ExternalInput")
    w_d = nc.dram_tensor("w", (TRGD, ENCD), F32R, kind="ExternalInput")
    encT_d = nc.dram_tensor("encT", (BPC, ENCD, SRC), F32R, kind="ExternalInput")
    val_d = nc.dram_tensor("val", (BPC, SRC, TRGD), BF16, kind="ExternalInput")
    mask_d = nc.dram_tensor("mask", (BPC, 1, SRC), F32, kind="ExternalInput")
    ae_d = nc.dram_tensor("ae", (BPC, TRG, SRC), BF16, kind="ExternalOutput")
    aw_d = nc.dram_tensor("aw", (BPC, TRG, SRC), BF16, kind="ExternalOutput")
    ctx_d = nc.dram_tensor("ctx", (BPC, TRG, TRGD), BF16, kind="ExternalOutput")

    with tile.TileContext(nc) as tc:
        with (
            tc.tile_pool(name="const", bufs=1) as const,
            tc.tile_pool(name="wp", bufs=1) as wp,
            tc.tile_pool(name="big", bufs=1) as big,
            tc.tile_pool(name="sm", bufs=2) as sm,
            tc.tile_pool(name="xs", bufs=4) as xs,
            tc.tile_pool(name="psA", bufs=2, space="PSUM") as psA,
            tc.tile_pool(name="psB", bufs=3, space="PSUM") as psB,
        ):
            ident = const.tile([P, P], F32)
            make_identity(nc, ident[:])
            identb = const.tile([P, P], BF16)
            nc.vector.tensor_copy(identb[:], ident[:])

            w_sb = [wp.tile([P, ENCD], F32R, tag=f"w{i}", name=f"w_sb{i}")
                    for i in range(nD)]

            def emit_loads(b):
                hidT_sb = [big.tile([P, TRG], F32R, tag=f"hidT{i}",
                                    name=f"hidT_sb{i}") for i in range(nD)]
                # DMA issue order == consumption order for the b0 ramp
                for i in range(nD):
                    if b == 0:
                        nc.sync.dma_start(out=w_sb[i][:],
                                          in_=w_d[i * P:(i + 1) * P, :])
                    nc.sync.dma_start(out=hidT_sb[i][:],
                                      in_=hidT_d[b, i * P:(i + 1) * P, :])
                maskb = sm.tile([P, SRC], F32, tag="maskb")
                nc.sync.dma_start(out=maskb[:],
                                  in_=mask_d[b].to_broadcast((P, SRC)))
                maskb_bf = sm.tile([P, SRC], BF16, tag="maskb_bf")
                nc.vector.tensor_copy(maskb_bf[:], maskb[:])
                encT_sb = big.tile([P, nE, SRC], F32R, tag="encT")
                for i in range(nE):
                    nc.sync.dma_start(out=encT_sb[:, i, :],
                                      in_=encT_d[b, i * P:(i + 1) * P, :])
                val_sb = big.tile([P, nS, TRGD], BF16, tag="val")
                for i in range(nS):
                    nc.sync.dma_start(out=val_sb[:, i, :],
                                      in_=val_d[b, i * P:(i + 1) * P, :])
                return hidT_sb, (maskb, maskb_bf), encT_sb, val_sb

            def emit_mm1_ramp(hidT_sb):
                """b0: dt-outer over 8 concurrent psum groups; each
                (w[dt], hidT[dt]) pair is consumed as its DMA lands; the
                et-outer second half staggers the HpT copies on DVE."""
                HpT = big.tile([P, nE, TRG], F32R, tag="HpT", name="HpT0")
                mm1_ps = [psB.tile([P, SRC], F32, tag="ps_b",
                                   name=f"mm1ps{j}") for j in range(3)]
                mm1_ps2 = [psA.tile([P, TRG], F32, tag="ps_a",
                                    name=f"mm1ps2{j}") for j in range(2)]

                def et_psum(et):
                    if et < 6:
                        return mm1_ps[et // 2][:, (et % 2) * 512:
                                               (et % 2 + 1) * 512]
                    return mm1_ps2[et - 6][:]

                for dt in range(nD // 2):
                    for et in range(nE):
                        nc.tensor.matmul(et_psum(et),
                                         w_sb[dt][:, et * P:(et + 1) * P],
                                         hidT_sb[dt][:],
                                         start=(dt == 0), stop=False)
                for et in range(nE):
                    for dt in range(nD // 2, nD):
                        nc.tensor.matmul(et_psum(et),
                                         w_sb[dt][:, et * P:(et + 1) * P],
                                         hidT_sb[dt][:],
                                         start=False, stop=(dt == nD - 1))
                    nc.vector.tensor_copy(HpT[:, et, :], et_psum(et))
                return HpT

            def mm1_chunks(hidT_sb):
                """b>0: et-outer groups (2 per chunk) on psB halves, meant to
                be interleaved into the previous batch's transpose/mm3 phase
                (which only holds one psB slot at a time)."""
                HpT = big.tile([P, nE, TRG], F32R, tag="HpT", name="HpT1")

                def chunk(et_pair):
                    def emit():
                        for et in et_pair:
                            pp = psB.tile([P, SRC], F32, tag="ps_b",
                                          name=f"mm1b_ps{et}")
                            half = pp[:, :TRG]
                            for dt in range(nD):
                                nc.tensor.matmul(
                                    half, w_sb[dt][:, et * P:(et + 1) * P],
                                    hidT_sb[dt][:],
                                    start=(dt == 0), stop=(dt == nD - 1))
                            nc.vector.tensor_copy(HpT[:, et, :], half)
                    return emit
                return HpT, [chunk((2 * j, 2 * j + 1)) for j in range(nE // 2)]

            def emit_mm2_chains(b, masks, HpT, encT_sb):
                maskb, maskb_bf = masks
                attns = []
                prev_last_mm = None
                for tt in range(nT):
                    ts = slice(tt * P, (tt + 1) * P)
                    en_ps = psB.tile([P, SRC], F32, tag="ps_b")
                    first_mm = None
                    for et in range(nE):
                        for h in range(2):
                            hs = slice(h * 512, (h + 1) * 512)
                            mm = nc.tensor.matmul(en_ps[:, hs], HpT[:, et, ts],
                                                  encT_sb[:, et, hs],
                                                  start=(et == 0),
                                                  stop=(et == nE - 1))
                            if first_mm is None:
                                first_mm = mm
                            last_mm = mm
                    # keep mm2 groups sequential on PE: otherwise the
                    # scheduler interleaves groups and delays group 0's stop
                    # (and with it every softmax chain) by ~8us.
                    if prev_last_mm is not None:
                        _add_dep(first_mm.ins, prev_last_mm.ins, sync=False,
                                 reason="mm2 group order")
                    prev_last_mm = last_mm

                    x = xs.tile([P, SRC], F32, tag="x")
                    nc.vector.tensor_mul(x[:], en_ps[:], maskb[:])
                    ae_bf = sm.tile([P, SRC], BF16, tag="ae_bf")
                    nc.scalar.copy(ae_bf[:], x[:])
                    nc.sync.dma_start(out=ae_d[b, ts, :], in_=ae_bf[:])
                    negm = sm.tile([P, 1], F32, tag="negm")
                    nc.vector.tensor_reduce(negm[:], x[:], axis=AXL.X,
                                            op=ALU.max, negate=True)
                    ex = sm.tile([P, SRC], BF16, tag="ex")
                    nc.scalar.activation(ex[:], x[:], ACT_EXP, bias=negm[:],
                                         scale=1.0)
                    rowsum = sm.tile([P, 1], F32, tag="rowsum")
                    nc.vector.scalar_tensor_tensor(ex[:], ex[:], 1.0,
                                                   maskb_bf[:],
                                                   op0=ALU.mult, op1=ALU.mult,
                                                   accum_out=rowsum[:])
                    z = sm.tile([P, 1], F32, tag="z")
                    nc.vector.tensor_scalar_add(z[:], rowsum[:], 1e-6)
                    rz = sm.tile([P, 1], F32, tag="rz")
                    nc.vector.reciprocal(rz[:], z[:])
                    attn = xs.tile([P, SRC], BF16, tag="attn")
                    nc.vector.tensor_scalar_mul(attn[:], ex[:], rz[:])
                    nc.sync.dma_start(out=aw_d[b, ts, :], in_=attn[:])
                    attns.append(attn)
                return attns

            def emit_tail(b, attns, val_sb, filler_chunks):
                """Per t-tile: PE transposes of attn + mm3; interleave the
                next batch's mm1 chunks between t-tiles."""
                for tt in range(nT):
                    ts = slice(tt * P, (tt + 1) * P)
                    attn = attns[tt]
                    attnT = sm.tile([P, nS, P], BF16, tag="attnT")
                    for st in range(nS):
                        pt = psA.tile([P, TRG], F32, tag="ps_a")
                        ptb = pt[:].bitcast(BF16)
                        nc.tensor.transpose(ptb[:, :P],
                                            attn[:, st * P:(st + 1) * P],
                                            identb[:])
                        if st % 2 == 0:
                            nc.vector.tensor_copy(attnT[:, st, :], ptb[:, :P])
                        else:
                            nc.scalar.copy(attnT[:, st, :], ptb[:, :P])

                    ctx_ps = psB.tile([P, TRGD], F32, tag="ps_b")
                    for st in range(nS):
                        for h in range(2):
                            hs = slice(h * 512, (h + 1) * 512)
                            nc.tensor.matmul(ctx_ps[:, hs], attnT[:, st, :],
                                             val_sb[:, st, hs],
                                             start=(st == 0),
                                             stop=(st == nS - 1))
                    ctx_sb = sm.tile([P, TRGD], BF16, tag="ctx_sb")
                    nc.scalar.copy(ctx_sb[:], ctx_ps[:])
                    nc.sync.dma_start(out=ctx_d[b, ts, :], in_=ctx_sb[:])

                    if filler_chunks:
                        filler_chunks.pop(0)()
                for ch in filler_chunks:
                    ch()

            # ---- two-batch pipeline ----
            hidT0, maskb0, encT0, val0 = emit_loads(0)
            HpT0 = emit_mm1_ramp(hidT0)
            attns0 = emit_mm2_chains(0, maskb0, HpT0, encT0)

            hidT1, maskb1, encT1, val1 = emit_loads(1)
            HpT1, chunks1 = mm1_chunks(hidT1)
            emit_tail(0, attns0, val0, chunks1)

            attns1 = emit_mm2_chains(1, maskb1, HpT1, encT1)
            emit_tail(1, attns1, val1, [])

    nc.compile()
    return nc


def kernel(hidden, encoder_outputs, encoder_value, encoder_mask, W):
    global LAST_EXEC_NS, LAST_RESULTS
    from concourse.bass_utils import run_bass_kernel_spmd

    if "nc" not in _cache:
        _cache["nc"] = _build()
    nc = _cache["nc"]

    hidden = np.ascontiguousarray(hidden, dtype=np.float32)
    encoder_outputs = np.ascontiguousarray(encoder_outputs, dtype=np.float32)
    encoder_value = np.ascontiguousarray(encoder_value, dtype=np.float32)
    encoder_mask = np.ascontiguousarray(encoder_mask, dtype=np.float32)
    W = np.ascontiguousarray(W, dtype=np.float32)

    in_maps = []
    for c in range(NCORES):
        sl = slice(c * BPC, (c + 1) * BPC)
        in_maps.append({
            "hidT": np.ascontiguousarray(hidden[sl].transpose(0, 2, 1)),
            "w": W,
            "encT": np.ascontiguousarray(encoder_outputs[sl].transpose(0, 2, 1)),
            "val": encoder_value[sl].astype(ml_dtypes.bfloat16),
            "mask": encoder_mask[sl][:, None, :],
        })

    trace = bool(int(os.environ.get("KERNEL_TRACE", "0")))
    res = run_bass_kernel_spmd(nc, in_maps, core_ids=list(range(NCORES)),
                               trace=trace)
    LAST_EXEC_NS = res.exec_time_ns
    LAST_RESULTS = res

    context = np.concatenate([res.results[c]["ctx"] for c in range(NCORES)],
                             axis=0).astype(np.float32)
    attn_weights = np.concatenate([res.results[c]["aw"] for c in range(NCORES)],
                                  axis=0).astype(np.float32)
    attn_energies = np.concatenate([res.results[c]["ae"] for c in range(NCORES)],
                                   axis=0).astype(np.float32)
    return context, attn_weights, attn_energies



# revision 6
# speedup vs baseline: 1.2159x; 1.2159x over previous
"""Trainium2 Bass kernel for nn_Attention (general-score attention with
masked softmax), data-parallel over batch across 8 NeuronCores.

Math (per batch), matching the reference exactly for {0,1} float masks:
    raw[t,s]  = sum_e (hidden @ W)[t,e] * enc[s,e]       (associativity trick:
                (hidden @ W) @ enc^T  ==  hidden @ (enc @ W^T)^T, saves 25%
                FLOPs and avoids materializing proj)
    attn_energies = raw * mask            (mask in {0,1} so mask^2 == mask)
    e = exp(x - max_s x) * mask
    attn = e / (sum_s e + 1e-6)
    context = attn @ enc_value

v2, built from HAM/trace analysis of v1 (156us):
  - The PE clock-gates to 1.2GHz after any ~3.4us idle window; warm issue
    rate is ~272ns per 512-wide f32r matmul vs ~472ns per 1024-wide 16-bit
    matmul (LDWEIGHTS mostly hides).  So: halve the instruction count with
    1024-wide fp16 moving operands and keep the PE continuously busy.
  - fp16 (not bf16) keeps softmax accuracy: measured end-to-end rel err
    ~2.4e-3 in emulation, same as the all-f32r v1.  All three gemms run
    fp16 with f32 PSUM accumulation; softmax internals stay f32.
  - mm1 fuses BOTH batches into one 1024-wide moving operand (hidT01) and
    consumes (w[dt], hidT01[dt]) DMA pairs as they land; fp16 halves the
    DMA-gated startup vs f32.
  - mm2/softmax runs over 8 supertiles (batch, t-tile) in one stream;
    tail transposes are pipelined one tile ahead of mm3 so softmax latency
    and the attnT copy hide behind the previous tile's mm3.
  - single PSUM pool: 4 bufs x [128,1024] f32 = all 8 banks, ring order
    arranged so no allocation ever waits; PE program order pinned with an
    explicit dep chain.
"""
import os

import numpy as np

B, TRG, SRC, ENCD, TRGD = 16, 512, 1024, 1024, 1024
NCORES = 8
BPC = B // NCORES  # batches per core
P = 128
nD = TRGD // P   # 8 contraction tiles over d
nE = ENCD // P   # 8 over e
nS = SRC // P    # 8 over s
nT = TRG // P    # 4 t-tiles per batch
TRG2 = BPC * TRG  # both batches fused along t: 1024

_cache = {}

LAST_EXEC_NS = None
LAST_RESULTS = None


def _build():
    import bass_rust
    import concourse.mybir as mybir
    import concourse.tile as tile
    from concourse import bacc
    from concourse.masks import make_identity

    _add_dep = bass_rust.add_dep_helper

    F32 = mybir.dt.float32
    FP16 = mybir.dt.float16
    ALU = mybir.AluOpType
    AXL = mybir.AxisListType
    ACT_EXP = mybir.ActivationFunctionType.Exp

    nc = bacc.Bacc("TRN2", target_bir_lowering=False, debug=False)

    hidT_d = nc.dram_tensor("hidT", (TRGD, TRG2), FP16, kind="ExternalInput")
    w_d = nc.dram_tensor("w", (TRGD, ENCD), FP16, kind="ExternalInput")
    encT_d = nc.dram_tensor("encT", (BPC, ENCD, SRC), FP16, kind="ExternalInput")
    val_d = nc.dram_tensor("val", (BPC, SRC, TRGD), FP16, kind="ExternalInput")
    mask_d = nc.dram_tensor("mask", (BPC, 1, SRC), F32, kind="ExternalInput")
    ae_d = nc.dram_tensor("ae", (BPC, TRG, SRC), FP16, kind="ExternalOutput")
    aw_d = nc.dram_tensor("aw", (BPC, TRG, SRC), FP16, kind="ExternalOutput")
    ctx_d = nc.dram_tensor("ctx", (BPC, TRG, TRGD), FP16, kind="ExternalOutput")

    with tile.TileContext(nc) as tc:
        with (
            tc.tile_pool(name="const", bufs=1) as const,
            tc.tile_pool(name="wp", bufs=1) as wp,
            tc.tile_pool(name="big", bufs=1) as big,
            tc.tile_pool(name="sm", bufs=2) as sm,
            tc.tile_pool(name="ps", bufs=4, space="PSUM") as psp,
        ):
            ident = const.tile([P, P], F32)
            make_identity(nc, ident[:])
            identh = const.tile([P, P], FP16)
            nc.vector.tensor_copy(identh[:], ident[:])

            # PE program order is pinned with an explicit linear chain so the
            # scheduler can never interleave accumulation groups or delay a
            # group's stop (v1 lost ~8us to that).
            pe_prev = [None]

            def chain(mm):
                if pe_prev[0] is not None:
                    _add_dep(mm.ins, pe_prev[0].ins, sync=False,
                             reason="pe order")
                pe_prev[0] = mm
                return mm

            # ---- loads (issue order == consumption order) ----
            w_sb = [wp.tile([P, ENCD], FP16, tag=f"w{i}", name=f"w_sb{i}")
                    for i in range(nD)]
            hidT_sb = [big.tile([P, TRG2], FP16, tag=f"hidT{i}",
                                name=f"hidT_sb{i}") for i in range(nD)]
            for i in range(nD):
                nc.sync.dma_start(out=w_sb[i][:], in_=w_d[i * P:(i + 1) * P, :])
                nc.sync.dma_start(out=hidT_sb[i][:],
                                  in_=hidT_d[i * P:(i + 1) * P, :])
            maskbs = []
            for b in range(BPC):
                maskb = sm.tile([P, SRC], F32, tag="maskb", name=f"maskb{b}")
                nc.sync.dma_start(out=maskb[:],
                                  in_=mask_d[b].to_broadcast((P, SRC)))
                maskb_hf = sm.tile([P, SRC], FP16, tag="maskb_hf",
                                   name=f"maskb_hf{b}")
                nc.vector.tensor_copy(maskb_hf[:], maskb[:])
                maskbs.append((maskb, maskb_hf))
            encT_sb = []
            val_sb = []
            for b in range(BPC):
                e_t = big.tile([P, nE, SRC], FP16, tag="encT", bufs=2,
                               name=f"encT_sb{b}")
                for i in range(nE):
                    nc.sync.dma_start(out=e_t[:, i, :],
                                      in_=encT_d[b, i * P:(i + 1) * P, :])
                v_t = big.tile([P, nS, TRGD], FP16, tag="val", bufs=2,
                               name=f"val_sb{b}")
                for i in range(nS):
                    nc.sync.dma_start(out=v_t[:, i, :],
                                      in_=val_d[b, i * P:(i + 1) * P, :])
                encT_sb.append(e_t)
                val_sb.append(v_t)

            # ---- mm1: HpT[e, t01] = sum_d W[d,e] * hidT01[d, t01] ----
            # two half-passes of 4 et each (4 psum bufs per pass); drains are
            # emitted right after each et's stop-matmul so the ring slot is
            # free before the next pass (or phase B) reaches it.
            HpT = big.tile([P, nE, TRG2], FP16, tag="HpT", name="HpT")
            drain_eng = [0]

            def emit_mm1_pass(ets):
                pps = [psp.tile([P, TRG2], F32, tag="ps", name=f"mm1ps{et}")
                       for et in ets]
                for dt in range(nD):
                    for i, et in enumerate(ets):
                        for h in range(2):
                            hs = slice(h * 512, (h + 1) * 512)
                            chain(nc.tensor.matmul(
                                pps[i][:, hs],
                                w_sb[dt][:, et * P:(et + 1) * P],
                                hidT_sb[dt][:, hs],
                                start=(dt == 0), stop=(dt == nD - 1)))
                        if dt == nD - 1:
                            if drain_eng[0] % 2 == 0:
                                nc.vector.tensor_copy(HpT[:, et, :], pps[i][:])
                            else:
                                nc.scalar.copy(HpT[:, et, :], pps[i][:])
                            drain_eng[0] += 1

            emit_mm1_pass(range(0, nE // 2))
            emit_mm1_pass(range(nE // 2, nE))

            # ---- mm2 + masked softmax over 8 supertiles (b, tt) ----
            tiles = [(b, tt) for b in range(BPC) for tt in range(nT)]
            attns = []
            for b, tt in tiles:
                ts = slice(b * TRG + tt * P, b * TRG + (tt + 1) * P)
                maskb, maskb_hf = maskbs[b]
                en_ps = psp.tile([P, SRC], F32, tag="ps", name=f"en{b}{tt}")
                for et in range(nE):
                    for h in range(2):
                        hs = slice(h * 512, (h + 1) * 512)
                        chain(nc.tensor.matmul(en_ps[:, hs], HpT[:, et, ts],
                                               encT_sb[b][:, et, hs],
                                               start=(et == 0),
                                               stop=(et == nE - 1)))
                x = sm.tile([P, SRC], F32, tag="x")
                nc.vector.tensor_mul(x[:], en_ps[:], maskb[:])
                ae_hf = sm.tile([P, SRC], FP16, tag="ae_hf")
                nc.scalar.copy(ae_hf[:], x[:])
                nc.sync.dma_start(out=ae_d[b, tt * P:(tt + 1) * P, :],
                                  in_=ae_hf[:])
                negm = sm.tile([P, 1], F32, tag="negm")
                nc.vector.tensor_reduce(negm[:], x[:], axis=AXL.X,
                                        op=ALU.max, negate=True)
                ex = sm.tile([P, SRC], FP16, tag="ex")
                nc.scalar.activation(ex[:], x[:], ACT_EXP, bias=negm[:],
                                     scale=1.0)
                rowsum = sm.tile([P, 1], F32, tag="rowsum")
                nc.vector.scalar_tensor_tensor(ex[:], ex[:], 1.0,
                                               maskb_hf[:],
                                               op0=ALU.mult, op1=ALU.mult,
                                               accum_out=rowsum[:])
                z = sm.tile([P, 1], F32, tag="z")
                nc.vector.tensor_scalar_add(z[:], rowsum[:], 1e-6)
                rz = sm.tile([P, 1], F32, tag="rz")
                nc.vector.reciprocal(rz[:], z[:])
                attn = sm.tile([P, SRC], FP16, tag="attn", bufs=7,
                               name=f"attn{b}{tt}")
                nc.vector.tensor_scalar_mul(attn[:], ex[:], rz[:])
                nc.sync.dma_start(out=aw_d[b, tt * P:(tt + 1) * P, :],
                                  in_=attn[:])
                attns.append(attn)

            # ---- tail: PE transpose (one bank per tile) + mm3, transposes
            # pipelined one tile ahead of mm3 ----
            attnTs = {}

            def emit_tr(k):
                attn = attns[k]
                trp = psp.tile([P, SRC], F32, tag="ps", name=f"tr{k}")
                trh = trp[:].bitcast(FP16)
                for st in range(nS):
                    chain(nc.tensor.transpose(trh[:, st * P:(st + 1) * P],
                                              attn[:, st * P:(st + 1) * P],
                                              identh[:]))
                attnT = sm.tile([P, SRC], FP16, tag="attnT", name=f"attnT{k}")
                nc.scalar.copy(attnT[:], trh[:, :SRC])
                attnTs[k] = attnT

            def emit_mm3(k):
                b, tt = tiles[k]
                attnT = attnTs.pop(k)
                ctx_ps = psp.tile([P, TRGD], F32, tag="ps", name=f"ctx{k}")
                for st in range(nS):
                    for h in range(2):
                        hs = slice(h * 512, (h + 1) * 512)
                        chain(nc.tensor.matmul(ctx_ps[:, hs],
                                               attnT[:, st * P:(st + 1) * P],
                                               val_sb[b][:, st, hs],
                                               start=(st == 0),
                                               stop=(st == nS - 1)))
                ctx_sb = sm.tile([P, TRGD], FP16, tag="ctx_sb")
                nc.scalar.copy(ctx_sb[:], ctx_ps[:])
                nc.sync.dma_start(out=ctx_d[b, tt * P:(tt + 1) * P, :],
                                  in_=ctx_sb[:])

            emit_tr(0)
            for k in range(len(tiles)):
                if k + 1 < len(tiles):
                    emit_tr(k + 1)
                emit_mm3(k)

    nc.compile()
    return nc


def kernel(hidden, encoder_outputs, encoder_value, encoder_mask, W):
    global LAST_EXEC_NS, LAST_RESULTS
    from concourse.bass_utils import run_bass_kernel_spmd

    if "nc" not in _cache:
        _cache["nc"] = _build()
    nc = _cache["nc"]

    hidden = np.ascontiguousarray(hidden, dtype=np.float32)
    encoder_outputs = np.ascontiguousarray(encoder_outputs, dtype=np.float32)
    encoder_value = np.ascontiguousarray(encoder_value, dtype=np.float32)
    encoder_mask = np.ascontiguousarray(encoder_mask, dtype=np.float32)
    W = np.ascontiguousarray(W, dtype=np.float32)

    w_hf = W.astype(np.float16)
    in_maps = []
    for c in range(NCORES):
        sl = slice(c * BPC, (c + 1) * BPC)
        hid2 = hidden[sl]  # (2, TRG, TRGD)
        hidT01 = np.concatenate([hid2[0].T, hid2[1].T], axis=1)
        in_maps.append({
            "hidT": np.ascontiguousarray(hidT01.astype(np.float16)),
            "w": w_hf,
            "encT": np.ascontiguousarray(
                encoder_outputs[sl].transpose(0, 2, 1).astype(np.float16)),
            "val": encoder_value[sl].astype(np.float16),
            "mask": encoder_mask[sl][:, None, :],
        })

    trace = bool(int(os.environ.get("KERNEL_TRACE", "0")))
    res = run_bass_kernel_spmd(nc, in_maps, core_ids=list(range(NCORES)),
                               trace=trace)
    LAST_EXEC_NS = res.exec_time_ns
    LAST_RESULTS = res

    context = np.concatenate([res.results[c]["ctx"] for c in range(NCORES)],
                             axis=0).astype(np.float32)
    attn_weights = np.concatenate([res.results[c]["aw"] for c in range(NCORES)],
                                  axis=0).astype(np.float32)
    attn_energies = np.concatenate([res.results[c]["ae"] for c in range(NCORES)],
                                   axis=0).astype(np.float32)
    return context, attn_weights, attn_energies


# revision 11
# speedup vs baseline: 1.4559x; 1.1974x over previous
"""Trainium2 Bass kernel for nn_Attention (general-score attention with
masked softmax), data-parallel over batch across 8 NeuronCores.

Math (per batch), matching the reference exactly for {0,1} float masks:
    raw[t,s]  = sum_e (hidden @ W)[t,e] * enc[s,e]       (associativity trick:
                (hidden @ W) @ enc^T  ==  hidden @ (enc @ W^T)^T, saves 25%
                FLOPs and avoids materializing proj)
    attn_energies = raw * mask            (mask in {0,1} so mask^2 == mask)
    e = exp(x - max_s x) * mask
    attn = e / (sum_s e + 1e-6)
    context = attn @ enc_value

v3, from HAM/trace analysis of v1 (156us) and v2 (128us):
  - All three gemms in fp16 with f32 PSUM accumulation (measured end-to-end
    rel err ~2.4e-3, same as all-f32r).  Warm PE issue rate is ~259ns per
    512-wide 16-bit matmul; the PE clock-gates to 1.2GHz after any ~3.4us
    idle window, so the whole kernel is one gap-free PE instruction chain.
  - mm1 fuses BOTH batches into one moving operand and consumes
    (w[dt], hidT01[dt]) DMA pairs as they land dt-outer; the last two dt
    rounds go et-wise with the PSUM->SBUF drain emitted right after each
    et's stop so the next pass never waits on a drain (v2 lost ~4us there).
  - 8 junk transposes of the identity warm the HAM clock gate during the
    ~9us DMA/preamble dead time, so mm1 runs at 2.4GHz almost from the start.
  - attnT copies go on GpSimd (idle engine) - in v2 they queued behind the
    Scalar softmax backlog at the B->C boundary (~1.7us stall).
  - ae+aw are packed into one [128,2048] tile and one DMA per tile; encT
    and val are marshaled partition-major on the host so each loads with 2
    DMAs per batch.  The Sync engine issues each dma_start serially at
    ~0.73us, so fewer+bigger transfers keep it off the critical path.
  - the final tile's ctx drain is split across DVE+Scalar with two DMA
    halves to shorten the end-of-kernel tail.
"""
import os

import numpy as np

B, TRG, SRC, ENCD, TRGD = 16, 512, 1024, 1024, 1024
NCORES = 8
BPC = B // NCORES  # batches per core
P = 128
nD = TRGD // P   # 8 contraction tiles over d
nE = ENCD // P   # 8 over e
nS = SRC // P    # 8 over s
nT = TRG // P    # 4 t-tiles per batch
TRG2 = BPC * TRG  # both batches fused along t: 1024

_cache = {}

LAST_EXEC_NS = None
LAST_RESULTS = None


def _build():
    import bass_rust
    import concourse.mybir as mybir
    import concourse.tile as tile
    from concourse import bacc
    from concourse.masks import make_identity

    _add_dep = bass_rust.add_dep_helper

    F32 = mybir.dt.float32
    FP16 = mybir.dt.float16
    ALU = mybir.AluOpType
    AXL = mybir.AxisListType
    ACT_EXP = mybir.ActivationFunctionType.Exp

    nc = bacc.Bacc("TRN2", target_bir_lowering=False, debug=False)

    hidT_d = nc.dram_tensor("hidT", (TRGD, TRG2), FP16, kind="ExternalInput")
    w_d = nc.dram_tensor("w", (TRGD, ENCD), FP16, kind="ExternalInput")
    encT_d = nc.dram_tensor("encT", (BPC, P, nE, SRC), FP16,
                            kind="ExternalInput")
    val_d = nc.dram_tensor("val", (BPC, P, nS, TRGD), FP16,
                           kind="ExternalInput")
    mask_d = nc.dram_tensor("mask", (BPC, 1, SRC), F32, kind="ExternalInput")
    aeaw_d = nc.dram_tensor("aeaw", (BPC, TRG, 2 * SRC), FP16,
                            kind="ExternalOutput")
    ctx_d = nc.dram_tensor("ctx", (BPC, TRG, TRGD), FP16,
                           kind="ExternalOutput")

    with tile.TileContext(nc) as tc:
        with (
            tc.tile_pool(name="const", bufs=1) as const,
            tc.tile_pool(name="wp", bufs=1) as wp,
            tc.tile_pool(name="big", bufs=1) as big,
            tc.tile_pool(name="sm", bufs=2) as sm,
            tc.tile_pool(name="ps", bufs=4, space="PSUM") as psp,
        ):
            ident = const.tile([P, P], F32)
            make_identity(nc, ident[:])
            identh = const.tile([P, P], FP16)
            nc.vector.tensor_copy(identh[:], ident[:])

            # PE program order is pinned with an explicit linear chain so the
            # scheduler can never interleave accumulation groups or delay a
            # group's stop.
            pe_prev = [None]

            def chain(mm):
                if pe_prev[0] is not None:
                    _add_dep(mm.ins, pe_prev[0].ins, sync=False,
                             reason="pe order")
                pe_prev[0] = mm
                return mm

            # ---- loads (issue order == consumption order) ----
            w_sb = [wp.tile([P, ENCD], FP16, tag=f"w{i}", name=f"w_sb{i}")
                    for i in range(nD)]
            hidT_sb = [big.tile([P, TRG2], FP16, tag=f"hidT{i}",
                                name=f"hidT_sb{i}") for i in range(nD)]
            for i in range(nD):
                nc.sync.dma_start(out=w_sb[i][:], in_=w_d[i * P:(i + 1) * P, :])
                nc.sync.dma_start(out=hidT_sb[i][:],
                                  in_=hidT_d[i * P:(i + 1) * P, :])
            maskbs = []
            for b in range(BPC):
                maskb = sm.tile([P, SRC], F32, tag="maskb", name=f"maskb{b}")
                nc.sync.dma_start(out=maskb[:],
                                  in_=mask_d[b].to_broadcast((P, SRC)))
                maskb_hf = sm.tile([P, SRC], FP16, tag="maskb_hf",
                                   name=f"maskb_hf{b}")
                nc.vector.tensor_copy(maskb_hf[:], maskb[:])
                maskbs.append((maskb, maskb_hf))
            encT_sb = []
            val_sb = []
            for b in range(BPC):
                e_t = big.tile([P, nE, SRC], FP16, tag="encT", bufs=2,
                               name=f"encT_sb{b}")
                for g in range(2):
                    gs = slice(g * (nE // 2), (g + 1) * (nE // 2))
                    nc.sync.dma_start(out=e_t[:, gs, :],
                                      in_=encT_d[b, :, gs, :])
                v_t = big.tile([P, nS, TRGD], FP16, tag="val", bufs=2,
                               name=f"val_sb{b}")
                for g in range(2):
                    gs = slice(g * (nS // 2), (g + 1) * (nS // 2))
                    nc.sync.dma_start(out=v_t[:, gs, :],
                                      in_=val_d[b, :, gs, :])
                encT_sb.append(e_t)
                val_sb.append(v_t)

            # ---- mm1: HpT[e, t01] = sum_d W[d,e] * hidT01[d, t01] ----
            # two half-passes of 4 et each (4 psum bufs per pass).  dt-outer
            # for DMA pair-wise consumption, but the last two dt rounds go
            # et-wise with the drain right after each stop so ring slots free
            # up staggered instead of all at the end.
            HpT = big.tile([P, nE, TRG2], FP16, tag="HpT", name="HpT")
            drain_eng = [0]

            def drain(dst, src):
                if drain_eng[0] % 2 == 0:
                    nc.vector.tensor_copy(dst, src)
                else:
                    nc.scalar.copy(dst, src)
                drain_eng[0] += 1

            def mm1_mm(pp, dt, et):
                for h in range(2):
                    hs = slice(h * 512, (h + 1) * 512)
                    chain(nc.tensor.matmul(
                        pp[:, hs], w_sb[dt][:, et * P:(et + 1) * P],
                        hidT_sb[dt][:, hs],
                        start=(dt == 0), stop=(dt == nD - 1)))

            def emit_mm1_pass(ets, warm=False):
                pps = [psp.tile([P, TRG2], F32, tag="ps", name=f"mm1ps{et}")
                       for et in ets]
                if warm:
                    # junk transposes of the identity: keep the PE busy
                    # during the DMA/preamble dead time so the HAM clock
                    # gate is warm (2.4GHz) when real work arrives.  The
                    # garbage psum is overwritten by mm1's start=True.
                    junk_view = pps[0][:].bitcast(FP16)
                    for _ in range(8):
                        chain(nc.tensor.matmul(
                            junk_view[:, 0:P], identh[:], identh[:],
                            is_transpose=True, skip_group_check=True))
                for dt in range(nD - 2):
                    for i, et in enumerate(ets):
                        mm1_mm(pps[i], dt, et)
                for i, et in enumerate(ets):
                    for dt in (nD - 2, nD - 1):
                        mm1_mm(pps[i], dt, et)
                    drain(HpT[:, et, :], pps[i][:])

            emit_mm1_pass(range(0, nE // 2), warm=True)
            emit_mm1_pass(range(nE // 2, nE))

            # ---- mm2 + masked softmax over 8 supertiles (b, tt) ----
            tiles = [(b, tt) for b in range(BPC) for tt in range(nT)]
            pks = []
            attnTs = {}

            def emit_tr(k):
                attn = pks[k][:, SRC:]
                trp = psp.tile([P, SRC], F32, tag="ps", name=f"tr{k}")
                trh = trp[:].bitcast(FP16)
                for st in range(nS):
                    chain(nc.tensor.transpose(trh[:, st * P:(st + 1) * P],
                                              attn[:, st * P:(st + 1) * P],
                                              identh[:]))
                attnT = sm.tile([P, SRC], FP16, tag="attnT", name=f"attnT{k}")
                nc.scalar.copy(attnT[:], trh[:, :SRC])
                attnTs[k] = attnT

            def emit_mm2(k):
                b, tt = tiles[k]
                ts = slice(b * TRG + tt * P, b * TRG + (tt + 1) * P)
                en_ps = psp.tile([P, SRC], F32, tag="ps", name=f"en{b}{tt}")
                for et in range(nE):
                    for h in range(2):
                        hs = slice(h * 512, (h + 1) * 512)
                        chain(nc.tensor.matmul(en_ps[:, hs], HpT[:, et, ts],
                                               encT_sb[b][:, et, hs],
                                               start=(et == 0),
                                               stop=(et == nE - 1)))
                return en_ps

            def emit_softmax(k, en_ps):
                b, tt = tiles[k]
                maskb, maskb_hf = maskbs[b]
                x = sm.tile([P, SRC], F32, tag="x")
                nc.vector.tensor_mul(x[:], en_ps[:], maskb[:])
                # packed [ae | attn] tile: one output DMA per supertile
                pk = sm.tile([P, 2 * SRC], FP16, tag="aeaw", bufs=7,
                             name=f"aeaw{b}{tt}")
                nc.scalar.copy(pk[:, :SRC], x[:])
                negm = sm.tile([P, 1], F32, tag="negm")
                nc.vector.tensor_reduce(negm[:], x[:], axis=AXL.X,
                                        op=ALU.max, negate=True)
                ex = sm.tile([P, SRC], FP16, tag="ex")
                nc.scalar.activation(ex[:], x[:], ACT_EXP, bias=negm[:],
                                     scale=1.0)
                rowsum = sm.tile([P, 1], F32, tag="rowsum")
                nc.vector.scalar_tensor_tensor(ex[:], ex[:], 1.0,
                                               maskb_hf[:],
                                               op0=ALU.mult, op1=ALU.mult,
                                               accum_out=rowsum[:])
                z = sm.tile([P, 1], F32, tag="z")
                nc.vector.tensor_scalar_add(z[:], rowsum[:], 1e-6)
                rz = sm.tile([P, 1], F32, tag="rz")
                nc.vector.reciprocal(rz[:], z[:])
                nc.vector.tensor_scalar_mul(pk[:, SRC:], ex[:], rz[:])
                nc.sync.dma_start(out=aeaw_d[b, tt * P:(tt + 1) * P, :],
                                  in_=pk[:])
                pks.append(pk)

            for k in range(len(tiles)):
                en_ps = emit_mm2(k)
                if k == len(tiles) - 1:
                    # emit tr(T0) BEFORE the last softmax's engine ops: its
                    # attnT copy then sits ahead of them in the Scalar FIFO
                    # instead of queuing behind (v2 lost ~1.7us there)
                    emit_tr(0)
                emit_softmax(k, en_ps)

            def emit_mm3(k):
                b, tt = tiles[k]
                attnT = attnTs.pop(k)
                last = (k == len(tiles) - 1)
                ctx_ps = psp.tile([P, TRGD], F32, tag="ps", name=f"ctx{k}")
                for st in range(nS):
                    for h in range(2):
                        hs = slice(h * 512, (h + 1) * 512)
                        chain(nc.tensor.matmul(ctx_ps[:, hs],
                                               attnT[:, st * P:(st + 1) * P],
                                               val_sb[b][:, st, hs],
                                               start=(st == 0),
                                               stop=(st == nS - 1)))
                ctx_sb = sm.tile([P, TRGD], FP16, tag="ctx_sb")
                rows = slice(tt * P, (tt + 1) * P)
                if last:
                    # split the final drain across two engines + two DMAs to
                    # shorten the end-of-kernel tail
                    nc.vector.tensor_copy(ctx_sb[:, :512], ctx_ps[:, :512])
                    nc.scalar.copy(ctx_sb[:, 512:], ctx_ps[:, 512:])
                    nc.sync.dma_start(out=ctx_d[b, rows, 0:512],
                                      in_=ctx_sb[:, :512])
                    nc.sync.dma_start(out=ctx_d[b, rows, 512:],
                                      in_=ctx_sb[:, 512:])
                else:
                    nc.scalar.copy(ctx_sb[:], ctx_ps[:])
                    nc.sync.dma_start(out=ctx_d[b, rows, :], in_=ctx_sb[:])

            for k in range(len(tiles)):
                if k + 1 < len(tiles):
                    emit_tr(k + 1)
                emit_mm3(k)

    nc.compile()
    return nc


def kernel(hidden, encoder_outputs, encoder_value, encoder_mask, W):
    global LAST_EXEC_NS, LAST_RESULTS
    from concourse.bass_utils import run_bass_kernel_spmd

    if "nc" not in _cache:
        _cache["nc"] = _build()
    nc = _cache["nc"]

    hidden = np.ascontiguousarray(hidden, dtype=np.float32)
    encoder_outputs = np.ascontiguousarray(encoder_outputs, dtype=np.float32)
    encoder_value = np.ascontiguousarray(encoder_value, dtype=np.float32)
    encoder_mask = np.ascontiguousarray(encoder_mask, dtype=np.float32)
    W = np.ascontiguousarray(W, dtype=np.float32)

    w_hf = W.astype(np.float16)
    in_maps = []
    for c in range(NCORES):
        sl = slice(c * BPC, (c + 1) * BPC)
        hid2 = hidden[sl]  # (2, TRG, TRGD)
        hidT01 = np.concatenate([hid2[0].T, hid2[1].T], axis=1)
        # partition-major marshaling: x_d[b, p, tile, free] so each batch
        # loads with 2 contiguous DMAs
        encT = encoder_outputs[sl].transpose(0, 2, 1)  # (BPC, ENCD, SRC)
        encT_pm = encT.reshape(BPC, nE, P, SRC).transpose(0, 2, 1, 3)
        val_pm = encoder_value[sl].reshape(BPC, nS, P, TRGD).transpose(
            0, 2, 1, 3)
        in_maps.append({
            "hidT": np.ascontiguousarray(hidT01.astype(np.float16)),
            "w": w_hf,
            "encT": np.ascontiguousarray(encT_pm.astype(np.float16)),
            "val": np.ascontiguousarray(val_pm.astype(np.float16)),
            "mask": encoder_mask[sl][:, None, :],
        })

    trace = bool(int(os.environ.get("KERNEL_TRACE", "0")))
    res = run_bass_kernel_spmd(nc, in_maps, core_ids=list(range(NCORES)),
                               trace=trace)
    LAST_EXEC_NS = res.exec_time_ns
    LAST_RESULTS = res

    aeaw = [res.results[c]["aeaw"] for c in range(NCORES)]
    context = np.concatenate([res.results[c]["ctx"] for c in range(NCORES)],
                             axis=0).astype(np.float32)
    attn_energies = np.concatenate(
        [a[:, :, :SRC] for a in aeaw], axis=0).astype(np.float32)
    attn_weights = np.concatenate(
        [a[:, :, SRC:] for a in aeaw], axis=0).astype(np.float32)
    return context, attn_weights, attn_energies
